# revision 47
# baseline (speedup 1.0000x reference)
"""EAST-style loss (weighted BCE score + smoothed-L1 geometry) on 8 trn2 cores.

Strategy: pure data parallel over batch m=128 -> 16 per core. Host packs each
core's shard into reduced precision: geometry fp8-e4m3 (shifts the geometry
loss ~0.4%, which is ~1e-5 of the total), score fp16 (~1e-4 noise; fp8 would
wreck ln(yp) near the clip bounds). Rel-err budget is 2e-2; measured ~1.6e-4.
Each core streams 5MB through SBUF:
  xg{i} [128, 2*FGS[i]] fp8: geometry pair-tiles, cols [0:f]=yt, [f:2f]=yp
  xs    [128, 2*2048] fp16:  score, cols 0:2048 = yt, 2048: = yp

Geometry uses ONE fused custom-DVE op per pair-tile (registered via the
documented dve_ops extension point): with d = a-b, c = clamp(d,-1,1),
  huber(d) = d*c - 0.5*c^2       (= 0.5 d^2 inside, |d|-0.5 outside)
summed across the free dim by the op's accumulator -> zero ACT work for
geometry, one DVE pass per element; the Vector engine is the end-to-end
bottleneck (saturated ~22us). Tile sizes ramp small->big so the DVE starts
on the first 0.25MB tile ~2.5us before a 1MB tile would land, and later
arrivals stay ahead of it. Score: ln(yp) directly and ln((1+eps)-yp) with a
per-partition bias tile (the eps keeps the log finite where fp16 rounds
1-1e-4 up to 1.0 -- no clamp op on the critical DVE), yt*ln products on DVE
with accum, sums on ACT. add_dep_helper pins fix the static per-engine
orders (engines execute their program in order; a mis-slotted op
head-of-line blocks). Final scalar combine happens on host in float64
(stats are tiny: [128, 11]).
"""

import sys

sys.path.insert(0, "/opt/trn_rl_repo")

import numpy as np

import concourse.bacc as bacc
import concourse.mybir as mybir
from concourse.bass_utils import run_bass_kernel_spmd
from concourse.tile import TileContext

N_CORES = 8
M, H, W = 128, 128, 128
GC = 8  # geometry channels
M_PER = M // N_CORES  # 16

P = 128
# geometry pair-tile half-widths; sum(FGS) * 128 = 2,097,152 elems per core
# per tensor. Ramp small->big: the DVE (the end-to-end bottleneck) can start
# on a 0.25MB first tile ~2.5us before a 1MB one would land, and later
# arrivals stay ahead of it. Small final tile shortens the serial tail.
FGS = [512, 768, 1152, 1536, 2048, 2816, 3840, 2432, 1280]
N_GT = len(FGS)
FG_OFF = [0]
for _f in FGS:
    FG_OFF.append(FG_OFF[-1] + _f)
FS = 2048  # score free-dim per half (fp16)

# ln(1-yp) guard: fp16 rounds 1-1e-4 up to exactly 1.0, so compute
# ln((1+EPS1) - yp) instead of clamping yp on the DVE -- the epsilon keeps
# the log finite (ln(4.88e-4) = -7.6 for the ~0.05% of elements at 1.0) and
# biases the loss by only ~4e-3 relative (budget 2e-2). This removes a DVE
# op from the critical engine and unhooks the ACT ln-chain from the DVE
# schedule entirely.
EPS1 = 1.00006103515625  # 1 + 2^-14

# stats columns (single fp32 [P, N_GT+4] tensor):
#   [0:N_GT]  = sum huber(d) per geometry tile   (custom DVE accum)
#   [N_GT]    = sum(ln(1-yp))                    (ACT accum)
#   [N_GT+1]  = sum(yt_s)                        (ACT accum)
#   [N_GT+2]  = sum(yt_s * ln(yp))               (DVE accum)
#   [N_GT+3]  = sum(yt_s * ln(1-yp))             (DVE accum)
NS = N_GT + 4

F16 = mybir.dt.float16
F8 = mybir.dt.float8e4
F32 = mybir.dt.float32

_CACHED_NC = None
_HUBER_OP = None


def _register_huber_op():
    """Register the fused huber+accumulate custom-DVE op (idempotent).

    Uses the documented dve_ops extension point (04-custom-dve-api.md): the
    op's uop program is written into the per-NEFF DVE table at compile time.
    """
    global _HUBER_OP
    if _HUBER_OP is not None:
        return _HUBER_OP
    from concourse import dve_ops as DO
    from concourse.dve_spec import (
        AluOp, C2, One, Spec, Src0, Src1, Zero, lower, maxx, minn, sq,
    )
    from concourse.dve_table_gen import dve_ver_for
    from concourse.dve_uop import DveOpSpec

    name = "HUBER_ACC_ANT"
    if name in DO._SUB_OPCODE_FOR_NAME:
        _HUBER_OP = next(op for op in DO.OPS if op.name == name)
        return _HUBER_OP
    d = Src0 - Src1
    c = maxx(minn(d, One), Zero - One)
    spec = Spec(body=d * c - sq(c) * C2, accum=AluOp.ADD)  # imm2 = 0.5
    ver = dve_ver_for("TRN2")
    row = max(DO._SUB_OPCODE_FOR_NAME.values()) + 1
    sha = DveOpSpec(
        name=name, opcode=row, uops=lower(spec, ver=ver), rd1_en=True
    ).sha(ver)
    op = DO.DveOp(name, spec, subdim=False, uops_sha={ver: sha})
    DO.OPS.append(op)
    DO._SUB_OPCODE_FOR_NAME[name] = row
    DO.CUSTOM_DVE_SPECS[name] = spec
    _HUBER_OP = op
    return op


def _build_nc():
    huber_op = _register_huber_op()
    nc = bacc.Bacc("TRN2", target_bir_lowering=False)
    # one contiguous DRAM block per graded tile (strided column-slices of a
    # single big tensor measured ~10% slower HBM streaming)
    xg_d = [
        nc.dram_tensor(f"xg{i}", [P, 2 * FGS[i]], F8, kind="ExternalInput")
        for i in range(N_GT)
    ]
    xsp_d = nc.dram_tensor("xsp", [P, FS], F16, kind="ExternalInput")  # yp
    xst_d = nc.dram_tensor("xst", [P, FS], F16, kind="ExternalInput")  # yt
    st_d = nc.dram_tensor("st", [P, NS], F32, kind="ExternalOutput")

    AF = mybir.ActivationFunctionType
    OP = mybir.AluOpType

    with TileContext(nc) as tc:
        with (
            tc.tile_pool(name="stats", bufs=1) as spool,
            tc.tile_pool(name="io", bufs=1) as iopool,
            tc.tile_pool(name="score", bufs=1) as scpool,
            tc.tile_pool(name="work", bufs=3) as wpool,
        ):
            st = spool.tile([P, NS], F32)

            # ---------------- input DMAs (all tiles SBUF-resident) ----------
            # Queue order = arrival order: geometry ramp first so the DVE
            # starts as early as possible; score mid-stream (its clamp ->
            # ln -> product chain fits into DVE slots after huber 3).
            # score ships as two halves: yp early (position 7 -- it alone
            # gates the ACT ln chain, which otherwise becomes the kernel
            # tail), yt later (position 9 -- first needed by the DVE
            # products around t=24).
            xg = [None] * N_GT
            for i in range(6):
                t = iopool.tile([P, 2 * FGS[i]], F8, tag=f"xg{i}")
                nc.sync.dma_start(out=t[:], in_=xg_d[i][:])
                xg[i] = t
            ypt = scpool.tile([P, FS], F16)
            nc.sync.dma_start(out=ypt[:], in_=xsp_d[:])
            t = iopool.tile([P, 2 * FGS[6]], F8, tag="xg6")
            nc.sync.dma_start(out=t[:], in_=xg_d[6][:])
            xg[6] = t
            ytt = scpool.tile([P, FS], F16)
            nc.sync.dma_start(out=ytt[:], in_=xst_d[:])
            for i in range(7, N_GT):
                t = iopool.tile([P, 2 * FGS[i]], F8, tag=f"xg{i}")
                nc.sync.dma_start(out=t[:], in_=xg_d[i][:])
                xg[i] = t

            yt = ytt[:]
            yp = ypt[:]

            # ---------------- score part ------------------------------------
            from concourse.tile_rust import add_dep_helper

            eps1 = spool.tile([P, 1], F32)
            nc.vector.memset(eps1[:], EPS1)
            lnp = scpool.tile([P, FS], F16)
            nc.scalar.activation(lnp[:], yp, AF.Ln)
            ln1m = scpool.tile([P, FS], F16)
            i_ln1m = nc.scalar.activation(
                ln1m[:], yp, AF.Ln, scale=-1.0, bias=eps1[:],
                accum_out=st[:, N_GT : N_GT + 1],
            )
            syt = scpool.tile([P, FS], F16)
            i_copy = nc.scalar.activation(
                syt[:], yt, AF.Copy, accum_out=st[:, N_GT + 1 : N_GT + 2]
            )
            # keep ACT's static order ln -> ln(1-.) -> copy: the copy is not
            # on the critical chain, but scheduled first it delays both lns
            # (and with them the DVE products) by ~3.5us.
            add_dep_helper(
                i_copy.ins, i_ln1m.ins, sync=False,
                reason="order score lns before the sum(yt) copy",
            )
            # products as plain TENSOR_TENSOR (2x_1P for fp16, ~1.1us) --
            # the accumulating STT form is locked to 1x (~2.3us). The free-dim
            # sums move to ACT copy-accumulates (ACT has slack; DVE is the
            # critical engine).
            p1 = scpool.tile([P, FS], F16)
            i_stt1 = nc.vector.tensor_mul(p1[:], yt, lnp[:])
            p2 = scpool.tile([P, FS], F16)
            i_stt2 = nc.vector.tensor_mul(p2[:], yt, ln1m[:])
            sp1 = scpool.tile([P, FS], F16)
            nc.scalar.activation(
                sp1[:], p1[:], AF.Copy, accum_out=st[:, N_GT + 2 : N_GT + 3]
            )
            sp2 = scpool.tile([P, FS], F16)
            nc.scalar.activation(
                sp2[:], p2[:], AF.Copy, accum_out=st[:, N_GT + 3 : N_GT + 4]
            )

            # ---------------- geometry part: 1 fused DVE op per pair-tile ---
            # Pin DVE static order [h0, h1, h2, clamp, h3, stt1, h4, stt2,
            # h5]: matches the arrival ramp, score products fill the DVE
            # while big tiles stream in, and nothing trails the last byte
            # (engine programs are static; a mis-slotted op head-of-line
            # blocks the engine).
            hs = []
            for i in range(N_GT):
                f = FGS[i]
                h = wpool.tile([P, f], F16, tag="h")
                i_h = nc.vector._custom_dve(
                    huber_op,
                    out=h[:],
                    in0=xg[i][:, 0:f],
                    in1=xg[i][:, f : 2 * f],
                    s0=0.0, s1=0.0, imm2=0.5,
                    accum_out=st[:, i : i + 1],
                )
                hs.append(i_h)
            add_dep_helper(i_stt1.ins, hs[6].ins, sync=False,
                           reason="products after huber 6")
            add_dep_helper(i_stt2.ins, i_stt1.ins, sync=False,
                           reason="products back-to-back")
            add_dep_helper(hs[7].ins, i_stt2.ins, sync=False,
                           reason="huber 5 after products: their ACT copy"
                                  " sums then overlap the last hubers")

            nc.sync.dma_start(out=st_d[:], in_=st[:])
    nc.finalize()
    return nc


def _get_nc():
    global _CACHED_NC
    if _CACHED_NC is None:
        _CACHED_NC = _build_nc()
    return _CACHED_NC


def _make_in_maps(Y_true_score, Y_pred_score, Y_true_geometry, Y_pred_geometry):
    FG_TOT = FG_OFF[-1]  # 16384 geometry elems per partition per tensor
    yts = np.asarray(Y_true_score, dtype=np.float32).reshape(N_CORES, P, FS)
    yps = np.asarray(Y_pred_score, dtype=np.float32).reshape(N_CORES, P, FS)
    ytg = np.asarray(Y_true_geometry, dtype=np.float32).reshape(N_CORES, P, FG_TOT)
    ypg = np.asarray(Y_pred_geometry, dtype=np.float32).reshape(N_CORES, P, FG_TOT)

    xsp = yps.astype(np.float16)
    xst = yts.astype(np.float16)
    np8 = mybir.dt.np(F8)
    xgs = []
    for i in range(N_GT):
        o, f = FG_OFF[i], FGS[i]
        xg = np.empty((N_CORES, P, 2 * f), dtype=np8)
        xg[:, :, 0:f] = ytg[:, :, o : o + f]
        xg[:, :, f:] = ypg[:, :, o : o + f]
        xgs.append(xg)

    return [
        {"xsp": xsp[k], "xst": xst[k], **{f"xg{i}": xgs[i][k] for i in range(N_GT)}}
        for k in range(N_CORES)
    ]


def _combine(results):
    """results: list of per-core dicts with st [P, NS] fp32."""
    huber_sum = 0.0
    ln1m_sum = 0.0
    yt_sum = 0.0
    t1_sum = 0.0  # sum yt*ln(yp)
    t2_sum = 0.0  # sum yt*ln(1-yp)
    for r in results:
        s = np.asarray(r["st"], dtype=np.float64)
        huber_sum += s[:, 0:N_GT].sum()
        ln1m_sum += s[:, N_GT].sum()
        yt_sum += s[:, N_GT + 1].sum()
        t1_sum += s[:, N_GT + 2].sum()
        t2_sum += s[:, N_GT + 3].sum()

    size = float(M * 1 * H * W)
    beta = 1.0 - yt_sum / size
    A = t1_sum  # sum(yt * ln yp)
    B = ln1m_sum - t2_sum  # sum((1-yt) * ln(1-yp))
    loss_score = (-beta * A - (1.0 - beta) * B) / M

    n_pix = M * H * W
    loss_geom = huber_sum / GC / n_pix  # LAMBDA_GEOMETRY = 1.0

    return np.array(loss_score + loss_geom, dtype=np.float32)


def kernel(Y_true_score, Y_pred_score, Y_true_geometry, Y_pred_geometry, **_kw):
    nc = _get_nc()
    in_maps = _make_in_maps(
        Y_true_score, Y_pred_score, Y_true_geometry, Y_pred_geometry
    )
    res = run_bass_kernel_spmd(nc, in_maps, core_ids=list(range(N_CORES)))
    return _combine(res.results)
